# revision 1
# baseline (speedup 1.0000x reference)
"""Bass/Trainium2 kernel for DeformableDETR-style loss, data-parallel over 8 cores.

Math notes (per element x of pred_logits):
  p = sigmoid(x), s = softplus(x), u = 1 - p = sigmoid(-x), nL = ln(u) = -s
  background focal  = 0.75 * p^2 * s              =: 0.75 * Phi(x)
  foreground focal  = 0.25 * (1-p)^2 * softplus(-x) = 0.25 * Phi(-x)
  Sum Phi = Sum (1-u)^2 * s = -[Sum nL - 2*Sum u*nL + Sum u^2*nL]
The device accumulates Sum nL (ACT accum), Sum u*nL and Sum u^2*nL
(scalar_tensor_tensor accum), per partition.  The scattered-match
corrections, box L1, paired GIoU and cardinality counts are also computed
on device; the host only gathers rows by index, sums partial accumulators
and applies the constant weights.
"""

import numpy as np

B, Q, C, Nt = 1024, 900, 8, 32
NCORES = 8
BPC = B // NCORES          # 128 batches per core = SBUF partitions
QC = Q * C                 # 7200 free elements per partition
NCH = 4                    # bulk chunks
CH = QC // NCH             # 1800
QCH = Q // NCH             # 225 query groups per chunk

ALPHA, GAMMA = 0.25, 2.0
EOS_COEF = 0.1
W_CE, W_BBOX, W_GIOU, W_CARD = 1.0, 5.0, 2.0, 1.0

# small-input column layout: xrow(256) | xstar(32) | -xstar(32) | aq | wq | sb(128) | tb(128) | lab
SM_XCAT = 0
SM_AQ = 320
SM_WQ = 352
SM_SB = 384
SM_TB = 512
SM_LAB = 640
SM_N = 672

# result column layout
R_ANL0, R_AW0, R_AW20 = 0, NCH, 2 * NCH          # per-chunk accums: [0:4), [4:8), [8:12)
R_CARD, R_AC1, R_AC2, R_ABB, R_AGIOU = 12, 13, 14, 15, 16
R_N = 17

_BULK_BF16 = True
_DEBUG = False

_nc_cache = {}


def _build_bass():
    import concourse.bass as bass
    from concourse import mybir

    F32 = mybir.dt.float32
    UD = mybir.dt.bfloat16 if _BULK_BF16 else mybir.dt.float32
    ALU = mybir.AluOpType
    ACTF = mybir.ActivationFunctionType

    nc = bass.Bass("TRN2", target_bir_lowering=False, debug=False,
                   num_devices=NCORES)
    xl = nc.dram_tensor("xl", [BPC, QC], F32, kind="ExternalInput")
    sm = nc.dram_tensor("sm", [BPC, SM_N], F32, kind="ExternalInput")
    res = nc.dram_tensor("res", [BPC, R_N], F32, kind="ExternalOutput")
    dbg = (nc.dram_tensor("dbg", [BPC, 832], F32, kind="ExternalOutput")
           if _DEBUG else None)

    def bcast4(ap32):
        # [128, 32] -> [128, 32, 4] via step-0 inner dim
        return bass.AP(tensor=ap32.tensor, offset=ap32.offset,
                       ap=[ap32.ap[0], list(ap32.ap[1]), [0, 4]])

    from contextlib import ExitStack
    with ExitStack() as ctx:
        e = ctx.enter_context
        xt = e(nc.sbuf_tensor([BPC, QC], F32))
        ut = e(nc.sbuf_tensor([BPC, QC], UD))
        nlt = e(nc.sbuf_tensor([BPC, QC], UD))
        wt = e(nc.sbuf_tensor([BPC, QC], UD))
        m1 = e(nc.sbuf_tensor([BPC, QC // 2], UD))
        m2 = e(nc.sbuf_tensor([BPC, QC // 4], UD))
        m3 = e(nc.sbuf_tensor([BPC, Q], UD))
        dum9 = e(nc.sbuf_tensor([BPC, Q], UD))
        smt = e(nc.sbuf_tensor([BPC, SM_N], F32))
        ucat = e(nc.sbuf_tensor([BPC, 320], F32))
        nlcat = e(nc.sbuf_tensor([BPC, 320], F32))
        usub = e(nc.sbuf_tensor([BPC, 320], F32))
        s2c = e(nc.sbuf_tensor([BPC, 320], F32))
        phin = e(nc.sbuf_tensor([BPC, 320], F32))
        ph8 = e(nc.sbuf_tensor([BPC, 32], F32))
        t2n = e(nc.sbuf_tensor([BPC, 32], F32))
        dd = e(nc.sbuf_tensor([BPC, 128], F32))
        ad = e(nc.sbuf_tensor([BPC, 128], F32))
        g1 = e(nc.sbuf_tensor([BPC, 32], F32))
        sc = e(nc.sbuf_tensor([BPC, 32], F32))
        hwa = e(nc.sbuf_tensor([BPC, 64], F32))
        hwb = e(nc.sbuf_tensor([BPC, 64], F32))
        axy = e(nc.sbuf_tensor([BPC, 128], F32))
        bxy = e(nc.sbuf_tensor([BPC, 128], F32))
        mxt = e(nc.sbuf_tensor([BPC, 128], F32))
        mnt = e(nc.sbuf_tensor([BPC, 128], F32))
        whi = e(nc.sbuf_tensor([BPC, 64], F32))
        whe = e(nc.sbuf_tensor([BPC, 64], F32))
        inter = e(nc.sbuf_tensor([BPC, 32], F32))
        dv64 = e(nc.sbuf_tensor([BPC, 64], F32))
        aab = e(nc.sbuf_tensor([BPC, 32], F32))
        abb = e(nc.sbuf_tensor([BPC, 32], F32))
        lnua = e(nc.sbuf_tensor([BPC, 64], F32))
        rec = e(nc.sbuf_tensor([BPC, 64], F32))
        iou = e(nc.sbuf_tensor([BPC, 32], F32))
        et1 = e(nc.sbuf_tensor([BPC, 32], F32))
        gneg = e(nc.sbuf_tensor([BPC, 32], F32))
        rest = e(nc.sbuf_tensor([BPC, R_N], F32))
        sd = e(nc.semaphore("sd"))
        sa = e(nc.semaphore("sa"))
        sv = e(nc.semaphore("sv"))
        block = e(nc.Block())
        smv = smt.ap()
        xcat = smv[:, SM_XCAT:SM_XCAT + 320]
        aq = smv[:, SM_AQ:SM_AQ + 32]
        wq = smv[:, SM_WQ:SM_WQ + 32]
        sb = smv[:, SM_SB:SM_SB + 128].rearrange("p (n c) -> p n c", c=4)
        tb = smv[:, SM_TB:SM_TB + 128].rearrange("p (n c) -> p n c", c=4)
        lab = smv[:, SM_LAB:SM_LAB + 32]

        # ---------------- DMA program ----------------
        @block.sync
        def _(sync):
            sync.dma_start(out=smt[:], in_=sm[:]).then_inc(sd, 16)
            for k in range(NCH):
                sync.dma_start(out=xt[:, k * CH:(k + 1) * CH],
                               in_=xl[:, k * CH:(k + 1) * CH]).then_inc(sd, 16)
            sync.wait_ge(sa, 2 + 2 * NCH)      # nl accums written (sa=10)
            sync.wait_ge(sv, 3)                # final DVE inc
            sync.dma_start(out=res[:], in_=rest[:]).then_inc(sd, 16)
            if dbg is not None:
                dv = dbg.ap()
                for lo, hi, t in [(0, 32, sc), (32, 160, ad), (160, 192, inter),
                                  (192, 256, dv64), (256, 320, rec),
                                  (320, 352, gneg), (352, 480, mxt),
                                  (480, 608, mnt), (608, 736, axy),
                                  (736, 800, hwa), (800, 832, g1)]:
                    sync.dma_start(out=dv[:, lo:hi], in_=t[:]).then_inc(sd, 16)

        # ---------------- ACT program ----------------
        # pass order keeps one activation-table set at a time:
        #   [sigmoid]: ucat, u_0..u_3   [ln/exp]: nlcat, nl_0..3, lnua, rec
        @block.scalar
        def _(scalar):
            scalar.wait_ge(sd, 16)
            nc.scalar.activation(out=ucat[:], in_=xcat, func=ACTF.Sigmoid,
                                 scale=-1.0).then_inc(sa, 1)                 # sa=1
            for k in range(NCH):
                scalar.wait_ge(sd, 32 + 16 * k)
                nc.scalar.activation(out=ut[:, k * CH:(k + 1) * CH],
                                     in_=xt[:, k * CH:(k + 1) * CH],
                                     func=ACTF.Sigmoid,
                                     scale=-1.0).then_inc(sa, 1)            # sa=2..5
            nc.scalar.activation(out=nlcat[:], in_=ucat[:],
                                 func=ACTF.Ln).then_inc(sa, 1)              # sa=6
            for k in range(NCH):
                nc.scalar.activation(out=nlt[:, k * CH:(k + 1) * CH],
                                     in_=ut[:, k * CH:(k + 1) * CH],
                                     func=ACTF.Ln,
                                     accum_out=rest[:, R_ANL0 + k:R_ANL0 + k + 1],
                                     ).then_inc(sa, 1)                      # sa=7..10
            scalar.wait_ge(sv, 1)   # dv64 ready (box prep)
            nc.scalar.activation(out=lnua[:], in_=dv64[:],
                                 func=ACTF.Ln).then_inc(sa, 1)              # sa=11
            nc.scalar.activation(out=rec[:], in_=lnua[:], func=ACTF.Exp,
                                 scale=-1.0).then_inc(sa, 1)                # sa=12

        # ---------------- DVE program ----------------
        @block.vector
        def _(vector):
            stt = nc.vector.scalar_tensor_tensor
            ts = nc.vector.tensor_scalar
            tt = nc.vector.tensor_tensor

            # --- box prep (needs only small DMA) ---
            vector.wait_ge(sd, 16)
            tt(out=dd[:], in0=sb, in1=tb, op=ALU.subtract)
            stt(out=ad[:], in0=dd[:], scalar=-1.0, in1=dd[:],
                op0=ALU.mult, op1=ALU.max)                       # |d|
            ts(out=g1[:], in0=lab, scalar1=4.0, scalar2=None, op0=ALU.is_ge)
            ts(out=iou[:], in0=lab, scalar1=6.0, scalar2=None, op0=ALU.is_le)
            nc.vector.drain()
            tt(out=et1[:], in0=g1[:], in1=iou[:], op=ALU.mult)   # rare mask
            nc.vector.drain()
            ts(out=sc[:], in0=et1[:], scalar1=1.0, scalar2=None, op0=ALU.add)
            nc.vector.drain()
            # Sum |d| * sc  (sc broadcast over the 4 box coords)
            stt(out=dd.ap().rearrange("p (n c) -> p n c", c=4),
                in0=ad.ap().rearrange("p (n c) -> p n c", c=4),
                scalar=1.0, in1=bcast4(sc.ap()), op0=ALU.mult, op1=ALU.mult,
                accum_out=rest[:, R_ABB:R_ABB + 1])
            # cxcywh -> xyxy for both box sets
            ts(out=hwa[:], in0=sb[:, :, 2:4], scalar1=0.5, scalar2=None, op0=ALU.mult)
            ts(out=hwb[:], in0=tb[:, :, 2:4], scalar1=0.5, scalar2=None, op0=ALU.mult)
            nc.vector.drain()
            h2a = hwa.ap().rearrange("p (n c) -> p n c", c=2)
            h2b = hwb.ap().rearrange("p (n c) -> p n c", c=2)
            tt(out=axy.ap()[:, 0:64].rearrange("p (n c) -> p n c", c=2),
               in0=sb[:, :, 0:2], in1=h2a, op=ALU.subtract)
            tt(out=axy.ap()[:, 64:128].rearrange("p (n c) -> p n c", c=2),
               in0=sb[:, :, 0:2], in1=h2a, op=ALU.add)
            tt(out=bxy.ap()[:, 0:64].rearrange("p (n c) -> p n c", c=2),
               in0=tb[:, :, 0:2], in1=h2b, op=ALU.subtract)
            tt(out=bxy.ap()[:, 64:128].rearrange("p (n c) -> p n c", c=2),
               in0=tb[:, :, 0:2], in1=h2b, op=ALU.add)
            tt(out=mxt[:], in0=axy[:], in1=bxy[:], op=ALU.max)   # [lt | rb_e]
            tt(out=mnt[:], in0=axy[:], in1=bxy[:], op=ALU.min)   # [lt_e | rb]
            tt(out=whi[:], in0=mnt.ap()[:, 64:128], in1=mxt.ap()[:, 0:64],
               op=ALU.subtract)
            nc.vector.drain()
            ts(out=whi[:], in0=whi[:], scalar1=0.0, scalar2=None, op0=ALU.max)
            nc.vector.drain()
            tt(out=whe[:], in0=mxt.ap()[:, 64:128], in1=mnt.ap()[:, 0:64],
               op=ALU.subtract)
            w2i = whi.ap().rearrange("p (n c) -> p n c", c=2)
            w2e = whe.ap().rearrange("p (n c) -> p n c", c=2)
            tt(out=inter[:], in0=w2i[:, :, 0], in1=w2i[:, :, 1], op=ALU.mult)
            tt(out=dv64.ap()[:, 32:64], in0=w2e[:, :, 0], in1=w2e[:, :, 1],
               op=ALU.mult)                                       # area_e
            tt(out=aab[:], in0=sb[:, :, 2], in1=sb[:, :, 3], op=ALU.mult)
            tt(out=abb[:], in0=tb[:, :, 2], in1=tb[:, :, 3], op=ALU.mult)
            tt(out=gneg[:], in0=aab[:], in1=abb[:], op=ALU.add)
            tt(out=dv64.ap()[:, 0:32], in0=gneg[:], in1=inter[:],
               op=ALU.subtract).then_inc(sv, 1)                   # union; sv=1

            # --- cardinality max-tree L1 per chunk ---
            for k in range(NCH):
                vector.wait_ge(sd, 32 + 16 * k)
                xg = xt.ap()[:, k * CH:(k + 1) * CH].rearrange(
                    "p (n c) -> p n c", c=8)
                tt(out=m1.ap()[:, k * CH // 2:(k + 1) * CH // 2].rearrange(
                    "p (n c) -> p n c", c=4),
                   in0=xg[:, :, 0:4], in1=xg[:, :, 4:8], op=ALU.max)
            m1g = m1.ap().rearrange("p (n c) -> p n c", c=4)
            tt(out=m2.ap().rearrange("p (n c) -> p n c", c=2),
               in0=m1g[:, :, 0:2], in1=m1g[:, :, 2:4], op=ALU.max)
            m2g = m2.ap().rearrange("p (n c) -> p n c", c=2)
            tt(out=m3[:], in0=m2g[:, :, 0], in1=m2g[:, :, 1], op=ALU.max)
            nc.vector.drain()
            ts(out=dum9[:], in0=m3[:], scalar1=0.0, scalar2=0.0,
               op0=ALU.is_gt, op1=ALU.add,
               accum_out=rest[:, R_CARD:R_CARD + 1])
            nc.vector.drain()

            # --- ce match corrections (need nlcat: sa>=6) ---
            vector.wait_ge(sa, 6)
            ts(out=usub[:], in0=ucat[:], scalar1=1.0, scalar2=None,
               op0=ALU.subtract)                                  # u-1 = -p
            nc.vector.drain()
            stt(out=s2c[:], in0=usub[:], scalar=1.0, in1=usub[:],
                op0=ALU.mult, op1=ALU.mult)                       # p^2
            stt(out=phin[:], in0=s2c[:], scalar=1.0, in1=nlcat[:],
                op0=ALU.mult, op1=ALU.mult)                       # -Phi
            nc.vector.drain()
            nc.vector.tensor_reduce(
                out=ph8[:], in_=phin.ap()[:, 0:256].rearrange(
                    "p (n c) -> p n c", c=8),
                axis=mybir.AxisListType.X, op=ALU.add)
            nc.vector.drain()
            stt(out=t2n[:], in0=ph8[:], scalar=1.0, in1=aq,
                op0=ALU.mult, op1=ALU.mult,
                accum_out=rest[:, R_AC1:R_AC1 + 1])
            stt(out=t2n[:], in0=phin.ap()[:, 288:320], scalar=1.0 / 3.0,
                in1=phin.ap()[:, 256:288], op0=ALU.mult, op1=ALU.subtract)
            stt(out=ph8[:], in0=t2n[:], scalar=1.0, in1=wq,
                op0=ALU.mult, op1=ALU.mult,
                accum_out=rest[:, R_AC2:R_AC2 + 1]).then_inc(sv, 1)  # sv=2

            # --- bulk focal accumulation per chunk ---
            for k in range(NCH):
                vector.wait_ge(sa, 7 + k)
                cs = slice(k * CH, (k + 1) * CH)
                stt(out=wt.ap()[:, cs], in0=ut.ap()[:, cs], scalar=1.0,
                    in1=nlt.ap()[:, cs], op0=ALU.mult, op1=ALU.mult,
                    accum_out=rest[:, R_AW0 + k:R_AW0 + k + 1])
                stt(out=wt.ap()[:, cs], in0=ut.ap()[:, cs], scalar=1.0,
                    in1=wt.ap()[:, cs], op0=ALU.mult, op1=ALU.mult,
                    accum_out=rest[:, R_AW20 + k:R_AW20 + k + 1])

            # --- giou finish (needs rec: sa>=12) ---
            vector.wait_ge(sa, 4 + 2 * NCH)
            tt(out=iou[:], in0=inter[:], in1=rec.ap()[:, 0:32], op=ALU.mult)
            tt(out=et1[:], in0=dv64.ap()[:, 32:64], in1=dv64.ap()[:, 0:32],
               op=ALU.subtract)
            tt(out=g1[:], in0=et1[:], in1=rec.ap()[:, 32:64], op=ALU.mult)
            stt(out=gneg[:], in0=iou[:], scalar=1.0, in1=g1[:],
                op0=ALU.subtract, op1=ALU.subtract)               # iou-1-eterm
            stt(out=aab[:], in0=gneg[:], scalar=1.0, in1=sc[:],
                op0=ALU.mult, op1=ALU.mult,
                accum_out=rest[:, R_AGIOU:R_AGIOU + 1]).then_inc(sv, 1)  # sv=3

    return nc


def _get_nc():
    if "nc" not in _nc_cache:
        _nc_cache["nc"] = _build_bass()
    return _nc_cache["nc"]


def _host_prep(pred_logits, pred_boxes, tgt_boxes, src_idx, tgt_labels,
               empty_weight):
    """Pure gather / index plumbing on the host."""
    si = np.asarray(src_idx).astype(np.int64)
    tl = np.asarray(tgt_labels).astype(np.int64)
    bidx = np.arange(B)[:, None]

    # gathers
    xrow = pred_logits[bidx, si]                    # [B, Nt, C]
    xstar = xrow[bidx, np.arange(Nt)[None, :], tl]  # [B, Nt]
    sboxes = pred_boxes[bidx, si]                   # [B, Nt, 4]

    # duplicate scatter emulation: last write wins per (b, q)
    # winner[b, n] = n is the last occurrence of si[b, n] within row b
    last_pos = np.full((B, Q), -1, dtype=np.int64)
    last_pos[bidx, si] = np.arange(Nt)[None, :]     # later n overwrites
    winner = last_pos[bidx, si] == np.arange(Nt)[None, :]

    ew = np.asarray(empty_weight, dtype=np.float32)
    wq = np.where(winner, ew[tl], 0.0).astype(np.float32)
    aq = np.where(winner, ew[tl] - EOS_COEF, 0.0).astype(np.float32)

    sm = np.empty((B, SM_N), dtype=np.float32)
    sm[:, SM_XCAT:SM_XCAT + 256] = xrow.reshape(B, 256)
    sm[:, SM_XCAT + 256:SM_XCAT + 288] = xstar
    sm[:, SM_XCAT + 288:SM_XCAT + 320] = -xstar
    sm[:, SM_AQ:SM_AQ + 32] = aq
    sm[:, SM_WQ:SM_WQ + 32] = wq
    sm[:, SM_SB:SM_SB + 128] = sboxes.reshape(B, 128)
    sm[:, SM_TB:SM_TB + 128] = np.asarray(tgt_boxes,
                                          dtype=np.float32).reshape(B, 128)
    sm[:, SM_LAB:SM_LAB + 32] = tl.astype(np.float32)
    return sm


def kernel(pred_logits, pred_boxes, tgt_boxes, src_idx, tgt_labels,
           empty_weight, _return_raw=False, _trace=False):
    from concourse.bass_utils import run_bass_kernel_spmd

    pred_logits = np.ascontiguousarray(np.asarray(pred_logits, dtype=np.float32))
    pred_boxes = np.asarray(pred_boxes, dtype=np.float32)
    tgt_boxes = np.asarray(tgt_boxes, dtype=np.float32)

    sm = _host_prep(pred_logits, pred_boxes, tgt_boxes, src_idx, tgt_labels,
                    empty_weight)
    xl = pred_logits.reshape(B, QC)

    in_maps = []
    for c in range(NCORES):
        rows = slice(c * BPC, (c + 1) * BPC)
        in_maps.append({"xl": np.ascontiguousarray(xl[rows]),
                        "sm": np.ascontiguousarray(sm[rows])})

    nc = _get_nc()
    out = run_bass_kernel_spmd(nc, in_maps, core_ids=list(range(NCORES)),
                               trace=_trace)
    r = np.concatenate([out.results[c]["res"] for c in range(NCORES)], axis=0)

    anl = r[:, R_ANL0:R_ANL0 + NCH].sum(dtype=np.float64)
    aw = r[:, R_AW0:R_AW0 + NCH].sum(dtype=np.float64)
    aw2 = r[:, R_AW20:R_AW20 + NCH].sum(dtype=np.float64)
    sum_phi = -anl + 2.0 * aw - aw2                  # Sum p^2 * softplus(x)

    ac1 = r[:, R_AC1].sum(dtype=np.float64)          # Sum aq * (-SumC Phi(row))
    ac2 = r[:, R_AC2].sum(dtype=np.float64)          # Sum wq * (-(Phi(-x*)/3 - Phi(x*)))
    ce_sum = (1.0 - ALPHA) * (EOS_COEF * sum_phi - ac1 - ac2)

    num_boxes = np.float32(B * Nt) + 1e-8
    loss_ce = ce_sum / num_boxes
    loss_bbox = r[:, R_ABB].sum(dtype=np.float64) / num_boxes
    loss_giou = -r[:, R_AGIOU].sum(dtype=np.float64) / num_boxes
    card = r[:, R_CARD]
    loss_card = np.abs(card - np.float32(Nt)).mean(dtype=np.float64)

    outv = np.array([W_CE * loss_ce, W_BBOX * loss_bbox,
                     W_GIOU * loss_giou, W_CARD * loss_card], dtype=np.float32)
    if _return_raw:
        return outv, r, out
    return outv



# revision 2
# speedup vs baseline: 3.6837x; 3.6837x over previous
"""Bass/Trainium2 kernel for DeformableDETR-style loss, data-parallel over 8 cores.

The axon tunnel (~70 MB/s) dominates wall time, so the design minimizes
bytes on the wire:

  * pred_logits are quantized host-side to int8 (scale 32, round-to-nearest,
    clip +-127) -> 7.4 MB upload instead of 29.5 MB f32.  The device computes
    the bulk background-focal sum and per-row cardinality counts from the
    quantized logits; the smooth focal sum has quantization bias ~1e-5 rel
    and the cardinality threshold shift (x>0 vs x>1/64) costs ~4e-4 rel.
  * All O(B*Nt) terms (box L1, paired GIoU, CE matched-query corrections)
    are computed on the host in float64 -- no `sm` side-input upload at all.
  * The jitted shard_map executable is built once and cached; the stock
    run_bass_kernel_spmd re-traces and re-lowers on every call.

Math notes (per element x of pred_logits):
  p = sigmoid(x), s = softplus(x), u = 1 - p = sigmoid(-x), ln u = -s
  background focal = 0.75 * p^2 * s =: 0.75 * Phi(x)
  Sum Phi = Sum (1-u)^2 * s = -[Sum ln u - 2*Sum u ln u + Sum u^2 ln u]
The device accumulates Sum ln u (ACT accum), Sum u ln u and Sum u^2 ln u
(scalar_tensor_tensor accum) per partition, plus per-partition counts of
max_c logit > 0 (min-tree over u, threshold mid-gap of the int8 lattice).
"""

import numpy as np

B, Q, C, Nt = 1024, 900, 8, 32
NCORES = 8
BPC = B // NCORES          # 128 batches per core = SBUF partitions
QC = Q * C                 # 7200 free elements per partition
NCH = 4                    # bulk chunks
CH = QC // NCH             # 1800

QSCALE = 32.0              # int8 quantization scale for logits
UTHRESH = 0.496            # sigmoid(-1/32)=0.49219 < t < sigmoid(0)=0.5

ALPHA = 0.25
EOS_COEF = 0.1
W_CE, W_BBOX, W_GIOU, W_CARD = 1.0, 5.0, 2.0, 1.0

# result column layout
R_ANL0, R_AW0, R_AW20 = 0, NCH, 2 * NCH          # per-chunk accums
R_CARD = 3 * NCH
R_N = 3 * NCH + 1

_cache = {}


def _build_bass():
    import concourse.bass as bass
    from concourse import mybir

    F32 = mybir.dt.float32
    BF16 = mybir.dt.bfloat16
    I8 = mybir.dt.int8
    ALU = mybir.AluOpType
    ACTF = mybir.ActivationFunctionType

    nc = bass.Bass("TRN2", target_bir_lowering=False, debug=False,
                   num_devices=NCORES)
    xq = nc.dram_tensor("xq", [BPC, QC], I8, kind="ExternalInput")
    res = nc.dram_tensor("res", [BPC, R_N], F32, kind="ExternalOutput")

    from contextlib import ExitStack
    with ExitStack() as ctx:
        e = ctx.enter_context
        xt = e(nc.sbuf_tensor([BPC, QC], I8))
        ut = e(nc.sbuf_tensor([BPC, QC], BF16))
        nlt = e(nc.sbuf_tensor([BPC, QC], BF16))
        wt = e(nc.sbuf_tensor([BPC, QC], BF16))
        m1 = e(nc.sbuf_tensor([BPC, QC // 2], BF16))
        m2 = e(nc.sbuf_tensor([BPC, QC // 4], BF16))
        m3 = e(nc.sbuf_tensor([BPC, Q], BF16))
        dum9 = e(nc.sbuf_tensor([BPC, Q], BF16))
        rest = e(nc.sbuf_tensor([BPC, R_N], F32))
        sd = e(nc.semaphore("sd"))
        sa = e(nc.semaphore("sa"))
        sv = e(nc.semaphore("sv"))
        block = e(nc.Block())

        # ---------------- DMA program ----------------
        @block.sync
        def _(sync):
            for k in range(NCH):
                sync.dma_start(out=xt[:, k * CH:(k + 1) * CH],
                               in_=xq[:, k * CH:(k + 1) * CH]).then_inc(sd, 16)
            sync.wait_ge(sa, 2 * NCH)   # anl accums written by ACT
            sync.wait_ge(sv, 2)         # card + bulk-mul accums written by DVE
            sync.dma_start(out=res[:], in_=rest[:]).then_inc(sd, 16)

        # ---------------- ACT program ----------------
        # one activation-table set at a time: all sigmoids, then all lns
        @block.scalar
        def _(scalar):
            for k in range(NCH):
                scalar.wait_ge(sd, 16 * (k + 1))
                nc.scalar.activation(out=ut[:, k * CH:(k + 1) * CH],
                                     in_=xt[:, k * CH:(k + 1) * CH],
                                     func=ACTF.Sigmoid,
                                     scale=-1.0 / QSCALE).then_inc(sa, 1)
            for k in range(NCH):
                nc.scalar.activation(out=nlt[:, k * CH:(k + 1) * CH],
                                     in_=ut[:, k * CH:(k + 1) * CH],
                                     func=ACTF.Ln,
                                     accum_out=rest[:, R_ANL0 + k:R_ANL0 + k + 1],
                                     ).then_inc(sa, 1)

        # ---------------- DVE program ----------------
        @block.vector
        def _(vector):
            stt = nc.vector.scalar_tensor_tensor
            ts = nc.vector.tensor_scalar
            tt = nc.vector.tensor_tensor

            # cardinality min-tree over u (min_c u <=> max_c x)
            for k in range(NCH):
                vector.wait_ge(sa, k + 1)
                ug = ut.ap()[:, k * CH:(k + 1) * CH].rearrange(
                    "p (n c) -> p n c", c=8)
                tt(out=m1.ap()[:, k * CH // 2:(k + 1) * CH // 2].rearrange(
                    "p (n c) -> p n c", c=4),
                   in0=ug[:, :, 0:4], in1=ug[:, :, 4:8], op=ALU.min)
            m1g = m1.ap().rearrange("p (n c) -> p n c", c=4)
            tt(out=m2.ap().rearrange("p (n c) -> p n c", c=2),
               in0=m1g[:, :, 0:2], in1=m1g[:, :, 2:4], op=ALU.min)
            m2g = m2.ap().rearrange("p (n c) -> p n c", c=2)
            tt(out=m3[:], in0=m2g[:, :, 0], in1=m2g[:, :, 1], op=ALU.min)
            nc.vector.drain()
            ts(out=dum9[:], in0=m3[:], scalar1=UTHRESH, scalar2=0.0,
               op0=ALU.is_lt, op1=ALU.add,
               accum_out=rest[:, R_CARD:R_CARD + 1]).then_inc(sv, 1)
            nc.vector.drain()

            # bulk focal accumulation per chunk
            for k in range(NCH):
                vector.wait_ge(sa, NCH + 1 + k)
                cs = slice(k * CH, (k + 1) * CH)
                stt(out=wt.ap()[:, cs], in0=ut.ap()[:, cs], scalar=1.0,
                    in1=nlt.ap()[:, cs], op0=ALU.mult, op1=ALU.mult,
                    accum_out=rest[:, R_AW0 + k:R_AW0 + k + 1])
                op = stt(out=wt.ap()[:, cs], in0=ut.ap()[:, cs], scalar=1.0,
                         in1=wt.ap()[:, cs], op0=ALU.mult, op1=ALU.mult,
                         accum_out=rest[:, R_AW20 + k:R_AW20 + k + 1])
                if k == NCH - 1:
                    op.then_inc(sv, 1)

    return nc


def _get_runner():
    """Build (once) the jitted 8-core shard_map executable for the NEFF."""
    if "runner" in _cache:
        return _cache["runner"]
    import jax
    from jax.sharding import Mesh, PartitionSpec
    from jax.experimental.shard_map import shard_map
    from concourse import mybir
    from concourse.bass2jax import (_bass_exec_p, install_neuronx_cc_hook,
                                    partition_id_tensor)

    nc = _build_bass()
    install_neuronx_cc_hook()

    partition_name = (nc.partition_id_tensor.name
                      if nc.partition_id_tensor else None)
    in_names, out_names, out_avals, zero_outs = [], [], [], []
    for alloc in nc.m.functions[0].allocations:
        if not isinstance(alloc, mybir.MemoryLocationSet):
            continue
        name = alloc.memorylocations[0].name
        if alloc.kind == "ExternalInput":
            if name != partition_name:
                in_names.append(name)
        elif alloc.kind == "ExternalOutput":
            shape = tuple(alloc.tensor_shape)
            dtype = mybir.dt.np(alloc.dtype)
            out_names.append(name)
            out_avals.append(jax.core.ShapedArray(shape, dtype))
            zero_outs.append(np.zeros(shape, dtype))
    n_params, n_outs = len(in_names), len(out_avals)
    in_names_all = list(in_names) + list(out_names)
    if partition_name is not None:
        in_names_all.append(partition_name)
    donate = tuple(range(n_params, n_params + n_outs))

    def _body(*args):
        operands = list(args)
        if partition_name is not None:
            operands.append(partition_id_tensor())
        outs = _bass_exec_p.bind(
            *operands, out_avals=tuple(out_avals),
            in_names=tuple(in_names_all), out_names=tuple(out_names),
            lowering_input_output_aliases=(), sim_require_finite=True,
            sim_require_nnan=True, nc=nc)
        return tuple(outs)

    devices = jax.devices()[:NCORES]
    mesh = Mesh(np.asarray(devices), ("core",))
    in_specs = (PartitionSpec("core"),) * (n_params + n_outs)
    out_specs = (PartitionSpec("core"),) * len(out_names)
    sharded = jax.jit(
        shard_map(_body, mesh=mesh, in_specs=in_specs, out_specs=out_specs,
                  check_rep=False),
        donate_argnums=donate, keep_unused=True)

    zero_shapes = [(NCORES * z.shape[0], *z.shape[1:]) for z in zero_outs]
    zero_dtypes = [z.dtype for z in zero_outs]
    _cache["runner"] = (sharded, in_names, zero_shapes, zero_dtypes)
    return _cache["runner"]


def _phi_bg(x):
    # p^2 * softplus(x), stable in float64
    p = 1.0 / (1.0 + np.exp(-x))
    s = np.maximum(x, 0.0) + np.log1p(np.exp(-np.abs(x)))
    return p * p * s


def _phi_fg(x):
    # (1-p)^2 * softplus(-x)
    u = 1.0 / (1.0 + np.exp(x))
    s = np.maximum(-x, 0.0) + np.log1p(np.exp(-np.abs(x)))
    return u * u * s


def _host_small_terms(pred_logits, pred_boxes, tgt_boxes, si, tl, ew):
    """CE matched-query correction, box L1 and paired GIoU sums (float64)."""
    bidx = np.arange(B)[:, None]

    # ---- CE correction over matched queries ----
    xrow = pred_logits[bidx, si].astype(np.float64)            # [B,Nt,C]
    phir = _phi_bg(xrow)
    phisum = phir.sum(axis=-1)                                 # [B,Nt]
    phistar = np.take_along_axis(phir, tl[..., None], 2)[..., 0]
    xstar = np.take_along_axis(xrow, tl[..., None], 2)[..., 0]
    phifg = _phi_fg(xstar)

    # duplicate scatter emulation: last write wins per (b, q)
    last_pos = np.full((B, Q), -1, dtype=np.int64)
    last_pos[bidx, si] = np.arange(Nt)[None, :]
    winner = last_pos[bidx, si] == np.arange(Nt)[None, :]

    ew_t = ew.astype(np.float64)[tl]                           # [B,Nt]
    corr_per = (ew_t * ((1.0 - ALPHA) * (phisum - phistar) + ALPHA * phifg)
                - EOS_COEF * (1.0 - ALPHA) * phisum)
    ce_corr = corr_per[winner].sum()

    # ---- box terms (all Nt entries, duplicates included, as in reference) ----
    a = pred_boxes[bidx, si].astype(np.float64)                # [B,Nt,4] cxcywh
    b = tgt_boxes.astype(np.float64)
    rare = (tl == 4) | (tl == 5) | (tl == 6)
    sc = np.where(rare, 2.0, 1.0)                              # [B,Nt]
    bbox_sum = (np.abs(a - b).sum(axis=-1) * sc).sum()

    ah, bh = 0.5 * a[..., 2:4], 0.5 * b[..., 2:4]
    a1, a2 = a[..., 0:2] - ah, a[..., 0:2] + ah                # xyxy
    b1, b2 = b[..., 0:2] - bh, b[..., 0:2] + bh
    lt = np.maximum(a1, b1)
    rb = np.minimum(a2, b2)
    wh = np.clip(rb - lt, 0.0, None)
    inter = wh[..., 0] * wh[..., 1]
    area_a = a[..., 2] * a[..., 3]
    area_b = b[..., 2] * b[..., 3]
    union = area_a + area_b - inter
    iou = inter / union
    lt_e = np.minimum(a1, b1)
    rb_e = np.maximum(a2, b2)
    wh_e = np.clip(rb_e - lt_e, 0.0, None)
    area_e = wh_e[..., 0] * wh_e[..., 1]
    giou = iou - (area_e - union) / area_e
    giou_sum = ((1.0 - giou) * sc).sum()

    return ce_corr, bbox_sum, giou_sum


def kernel(pred_logits, pred_boxes, tgt_boxes, src_idx, tgt_labels,
           empty_weight):
    pred_logits = np.asarray(pred_logits, dtype=np.float32)
    pred_boxes = np.asarray(pred_boxes, dtype=np.float32)
    tgt_boxes = np.asarray(tgt_boxes, dtype=np.float32)
    si = np.asarray(src_idx).astype(np.int64)
    tl = np.asarray(tgt_labels).astype(np.int64)
    ew = np.asarray(empty_weight, dtype=np.float32)

    sharded, in_names, zero_shapes, zero_dtypes = _get_runner()

    # quantize logits to int8 and dispatch to the 8 cores (async under jax)
    y = pred_logits.reshape(B, QC) * QSCALE
    np.rint(y, out=y)
    np.clip(y, -127.0, 127.0, out=y)
    xq = y.astype(np.int8)

    zeros = [np.zeros(s, d) for s, d in zip(zero_shapes, zero_dtypes)]
    out_arrs = sharded(xq, *zeros)

    # overlap: host small terms while the upload/exec round-trips
    ce_corr, bbox_sum, giou_sum = _host_small_terms(
        pred_logits, pred_boxes, tgt_boxes, si, tl, ew)

    r = np.asarray(out_arrs[0])                                # [B, R_N]

    anl = r[:, R_ANL0:R_ANL0 + NCH].sum(dtype=np.float64)
    aw = r[:, R_AW0:R_AW0 + NCH].sum(dtype=np.float64)
    aw2 = r[:, R_AW20:R_AW20 + NCH].sum(dtype=np.float64)
    sum_phi = -anl + 2.0 * aw - aw2                 # Sum p^2 * softplus(x)

    num_boxes = np.float32(B * Nt) + 1e-8
    ce_sum = EOS_COEF * (1.0 - ALPHA) * sum_phi + ce_corr
    loss_ce = ce_sum / num_boxes
    loss_bbox = bbox_sum / num_boxes
    loss_giou = giou_sum / num_boxes
    card = r[:, R_CARD]
    loss_card = np.abs(card - np.float32(Nt)).mean(dtype=np.float64)

    return np.array([W_CE * loss_ce, W_BBOX * loss_bbox,
                     W_GIOU * loss_giou, W_CARD * loss_card], dtype=np.float32)


# revision 4
# speedup vs baseline: 4.5862x; 1.2450x over previous
"""Bass/Trainium2 kernel for DeformableDETR-style loss, data-parallel over 8 cores.

The axon tunnel (~70 MB/s) dominates wall time, so the design minimizes
bytes on the wire:

  * pred_logits are quantized host-side to int8 (scale 32, round-to-nearest,
    clip +-127) -> 7.4 MB upload instead of 29.5 MB f32.  The device computes
    the bulk background-focal sum and per-row cardinality counts from the
    quantized logits; the smooth focal sum has quantization bias ~1e-5 rel
    and the cardinality threshold shift (x>0 vs x>1/64) costs ~4e-4 rel.
  * All O(B*Nt) terms (box L1, paired GIoU, CE matched-query corrections)
    are computed on the host in float64 -- no `sm` side-input upload at all.
  * The jitted shard_map executable is built once and cached; the stock
    run_bass_kernel_spmd re-traces and re-lowers on every call.

Math notes (per element x of pred_logits):
  p = sigmoid(x), s = softplus(x), u = 1 - p = sigmoid(-x), ln u = -s
  background focal = 0.75 * p^2 * s =: 0.75 * Phi(x)
  Sum Phi = Sum (1-u)^2 * s = -[Sum ln u - 2*Sum u ln u + Sum u^2 ln u]
The device accumulates Sum ln u (ACT accum), Sum u ln u and Sum u^2 ln u
(scalar_tensor_tensor accum) per partition, plus per-partition counts of
max_c logit > 0 (min-tree over u, threshold mid-gap of the int8 lattice).
"""

import numpy as np

B, Q, C, Nt = 1024, 900, 8, 32
NCORES = 8
BPC = B // NCORES          # 128 batches per core = SBUF partitions
QC = Q * C                 # 7200 free elements per partition
NCH = 4                    # bulk chunks
CH = QC // NCH             # 1800

QSCALE = 32.0              # int8 quantization scale for logits
UTHRESH = 0.496            # sigmoid(-1/32)=0.49219 < t < sigmoid(0)=0.5

ALPHA = 0.25
EOS_COEF = 0.1
W_CE, W_BBOX, W_GIOU, W_CARD = 1.0, 5.0, 2.0, 1.0

# result column layout
R_ANL0, R_AW0, R_AW20 = 0, NCH, 2 * NCH          # per-chunk accums
R_CARD = 3 * NCH
R_N = 3 * NCH + 1

_cache = {}


def _build_bass():
    import concourse.bass as bass
    from concourse import mybir

    F32 = mybir.dt.float32
    BF16 = mybir.dt.bfloat16
    I8 = mybir.dt.int8
    ALU = mybir.AluOpType
    ACTF = mybir.ActivationFunctionType

    nc = bass.Bass("TRN2", target_bir_lowering=False, debug=False,
                   num_devices=NCORES)
    xq = nc.dram_tensor("xq", [BPC, QC], I8, kind="ExternalInput")
    res = nc.dram_tensor("res", [BPC, R_N], F32, kind="ExternalOutput")

    from contextlib import ExitStack
    with ExitStack() as ctx:
        e = ctx.enter_context
        xt = e(nc.sbuf_tensor([BPC, QC], I8))
        ut = e(nc.sbuf_tensor([BPC, QC], BF16))
        nlt = e(nc.sbuf_tensor([BPC, QC], BF16))
        wt = e(nc.sbuf_tensor([BPC, QC], BF16))
        m1 = e(nc.sbuf_tensor([BPC, QC // 2], BF16))
        m2 = e(nc.sbuf_tensor([BPC, QC // 4], BF16))
        m3 = e(nc.sbuf_tensor([BPC, Q], BF16))
        dum9 = e(nc.sbuf_tensor([BPC, Q], BF16))
        rest = e(nc.sbuf_tensor([BPC, R_N], F32))
        sd = e(nc.semaphore("sd"))
        sa = e(nc.semaphore("sa"))
        sv = e(nc.semaphore("sv"))
        block = e(nc.Block())

        # ---------------- DMA program ----------------
        @block.sync
        def _(sync):
            for k in range(NCH):
                sync.dma_start(out=xt[:, k * CH:(k + 1) * CH],
                               in_=xq[:, k * CH:(k + 1) * CH]).then_inc(sd, 16)
            sync.wait_ge(sa, 2 * NCH)   # anl accums written by ACT
            sync.wait_ge(sv, 2)         # card + bulk-mul accums written by DVE
            sync.dma_start(out=res[:], in_=rest[:]).then_inc(sd, 16)

        # ---------------- ACT program ----------------
        # one activation-table set at a time: all sigmoids, then all lns
        @block.scalar
        def _(scalar):
            for k in range(NCH):
                scalar.wait_ge(sd, 16 * (k + 1))
                nc.scalar.activation(out=ut[:, k * CH:(k + 1) * CH],
                                     in_=xt[:, k * CH:(k + 1) * CH],
                                     func=ACTF.Sigmoid,
                                     scale=-1.0 / QSCALE).then_inc(sa, 1)
            for k in range(NCH):
                nc.scalar.activation(out=nlt[:, k * CH:(k + 1) * CH],
                                     in_=ut[:, k * CH:(k + 1) * CH],
                                     func=ACTF.Ln,
                                     accum_out=rest[:, R_ANL0 + k:R_ANL0 + k + 1],
                                     ).then_inc(sa, 1)

        # ---------------- DVE program ----------------
        @block.vector
        def _(vector):
            stt = nc.vector.scalar_tensor_tensor
            ts = nc.vector.tensor_scalar
            tt = nc.vector.tensor_tensor

            # cardinality min-tree over u (min_c u <=> max_c x)
            for k in range(NCH):
                vector.wait_ge(sa, k + 1)
                ug = ut.ap()[:, k * CH:(k + 1) * CH].rearrange(
                    "p (n c) -> p n c", c=8)
                tt(out=m1.ap()[:, k * CH // 2:(k + 1) * CH // 2].rearrange(
                    "p (n c) -> p n c", c=4),
                   in0=ug[:, :, 0:4], in1=ug[:, :, 4:8], op=ALU.min)
            m1g = m1.ap().rearrange("p (n c) -> p n c", c=4)
            tt(out=m2.ap().rearrange("p (n c) -> p n c", c=2),
               in0=m1g[:, :, 0:2], in1=m1g[:, :, 2:4], op=ALU.min)
            m2g = m2.ap().rearrange("p (n c) -> p n c", c=2)
            tt(out=m3[:], in0=m2g[:, :, 0], in1=m2g[:, :, 1], op=ALU.min)
            nc.vector.drain()
            ts(out=dum9[:], in0=m3[:], scalar1=UTHRESH, scalar2=0.0,
               op0=ALU.is_lt, op1=ALU.add,
               accum_out=rest[:, R_CARD:R_CARD + 1]).then_inc(sv, 1)
            nc.vector.drain()

            # bulk focal accumulation per chunk
            for k in range(NCH):
                vector.wait_ge(sa, NCH + 1 + k)
                cs = slice(k * CH, (k + 1) * CH)
                stt(out=wt.ap()[:, cs], in0=ut.ap()[:, cs], scalar=1.0,
                    in1=nlt.ap()[:, cs], op0=ALU.mult, op1=ALU.mult,
                    accum_out=rest[:, R_AW0 + k:R_AW0 + k + 1])
                op = stt(out=wt.ap()[:, cs], in0=ut.ap()[:, cs], scalar=1.0,
                         in1=wt.ap()[:, cs], op0=ALU.mult, op1=ALU.mult,
                         accum_out=rest[:, R_AW20 + k:R_AW20 + k + 1])
                if k == NCH - 1:
                    op.then_inc(sv, 1)

    return nc


def _get_runner():
    """Build (once) the jitted 8-core shard_map executable for the NEFF."""
    if "runner" in _cache:
        return _cache["runner"]
    import jax
    from jax.sharding import Mesh, PartitionSpec
    from jax.experimental.shard_map import shard_map
    from concourse import mybir
    from concourse.bass2jax import (_bass_exec_p, install_neuronx_cc_hook,
                                    partition_id_tensor)

    nc = _build_bass()
    install_neuronx_cc_hook()

    partition_name = (nc.partition_id_tensor.name
                      if nc.partition_id_tensor else None)
    in_names, out_names, out_avals, zero_outs = [], [], [], []
    for alloc in nc.m.functions[0].allocations:
        if not isinstance(alloc, mybir.MemoryLocationSet):
            continue
        name = alloc.memorylocations[0].name
        if alloc.kind == "ExternalInput":
            if name != partition_name:
                in_names.append(name)
        elif alloc.kind == "ExternalOutput":
            shape = tuple(alloc.tensor_shape)
            dtype = mybir.dt.np(alloc.dtype)
            out_names.append(name)
            out_avals.append(jax.core.ShapedArray(shape, dtype))
            zero_outs.append(np.zeros(shape, dtype))
    n_params, n_outs = len(in_names), len(out_avals)
    in_names_all = list(in_names) + list(out_names)
    if partition_name is not None:
        in_names_all.append(partition_name)
    donate = tuple(range(n_params, n_params + n_outs))

    def _body(*args):
        operands = list(args)
        if partition_name is not None:
            operands.append(partition_id_tensor())
        outs = _bass_exec_p.bind(
            *operands, out_avals=tuple(out_avals),
            in_names=tuple(in_names_all), out_names=tuple(out_names),
            lowering_input_output_aliases=(), sim_require_finite=True,
            sim_require_nnan=True, nc=nc)
        return tuple(outs)

    devices = jax.devices()[:NCORES]
    mesh = Mesh(np.asarray(devices), ("core",))
    in_specs = (PartitionSpec("core"),) * (n_params + n_outs)
    out_specs = (PartitionSpec("core"),) * len(out_names)
    sharded = jax.jit(
        shard_map(_body, mesh=mesh, in_specs=in_specs, out_specs=out_specs,
                  check_rep=False),
        donate_argnums=donate, keep_unused=True)

    from jax.sharding import NamedSharding
    xq_sharding = NamedSharding(mesh, PartitionSpec("core"))
    zero_shapes = [(NCORES * z.shape[0], *z.shape[1:]) for z in zero_outs]
    zero_dtypes = [z.dtype for z in zero_outs]
    _cache["runner"] = (sharded, devices, xq_sharding, zero_shapes, zero_dtypes)
    return _cache["runner"]


def _phi_bg(x):
    # p^2 * softplus(x), stable in float64
    p = 1.0 / (1.0 + np.exp(-x))
    s = np.maximum(x, 0.0) + np.log1p(np.exp(-np.abs(x)))
    return p * p * s


def _phi_fg(x):
    # (1-p)^2 * softplus(-x)
    u = 1.0 / (1.0 + np.exp(x))
    s = np.maximum(-x, 0.0) + np.log1p(np.exp(-np.abs(x)))
    return u * u * s


def _host_small_terms(pred_logits, pred_boxes, tgt_boxes, si, tl, ew):
    """CE matched-query correction, box L1 and paired GIoU sums (float64)."""
    bidx = np.arange(B)[:, None]

    # ---- CE correction over matched queries ----
    xrow = pred_logits[bidx, si].astype(np.float64)            # [B,Nt,C]
    phir = _phi_bg(xrow)
    phisum = phir.sum(axis=-1)                                 # [B,Nt]
    phistar = np.take_along_axis(phir, tl[..., None], 2)[..., 0]
    xstar = np.take_along_axis(xrow, tl[..., None], 2)[..., 0]
    phifg = _phi_fg(xstar)

    # duplicate scatter emulation: last write wins per (b, q)
    last_pos = np.full((B, Q), -1, dtype=np.int64)
    last_pos[bidx, si] = np.arange(Nt)[None, :]
    winner = last_pos[bidx, si] == np.arange(Nt)[None, :]

    ew_t = ew.astype(np.float64)[tl]                           # [B,Nt]
    corr_per = (ew_t * ((1.0 - ALPHA) * (phisum - phistar) + ALPHA * phifg)
                - EOS_COEF * (1.0 - ALPHA) * phisum)
    ce_corr = corr_per[winner].sum()

    # ---- box terms (all Nt entries, duplicates included, as in reference) ----
    a = pred_boxes[bidx, si].astype(np.float64)                # [B,Nt,4] cxcywh
    b = tgt_boxes.astype(np.float64)
    rare = (tl == 4) | (tl == 5) | (tl == 6)
    sc = np.where(rare, 2.0, 1.0)                              # [B,Nt]
    bbox_sum = (np.abs(a - b).sum(axis=-1) * sc).sum()

    ah, bh = 0.5 * a[..., 2:4], 0.5 * b[..., 2:4]
    a1, a2 = a[..., 0:2] - ah, a[..., 0:2] + ah                # xyxy
    b1, b2 = b[..., 0:2] - bh, b[..., 0:2] + bh
    lt = np.maximum(a1, b1)
    rb = np.minimum(a2, b2)
    wh = np.clip(rb - lt, 0.0, None)
    inter = wh[..., 0] * wh[..., 1]
    area_a = a[..., 2] * a[..., 3]
    area_b = b[..., 2] * b[..., 3]
    union = area_a + area_b - inter
    iou = inter / union
    lt_e = np.minimum(a1, b1)
    rb_e = np.maximum(a2, b2)
    wh_e = np.clip(rb_e - lt_e, 0.0, None)
    area_e = wh_e[..., 0] * wh_e[..., 1]
    giou = iou - (area_e - union) / area_e
    giou_sum = ((1.0 - giou) * sc).sum()

    return ce_corr, bbox_sum, giou_sum


def kernel(pred_logits, pred_boxes, tgt_boxes, src_idx, tgt_labels,
           empty_weight):
    pred_logits = np.asarray(pred_logits, dtype=np.float32)
    pred_boxes = np.asarray(pred_boxes, dtype=np.float32)
    tgt_boxes = np.asarray(tgt_boxes, dtype=np.float32)
    si = np.asarray(src_idx).astype(np.int64)
    tl = np.asarray(tgt_labels).astype(np.int64)
    ew = np.asarray(empty_weight, dtype=np.float32)

    import jax
    sharded, devices, xq_sharding, zero_shapes, zero_dtypes = _get_runner()

    # quantize logits to int8 per core slab; device_put is async, so slab c
    # streams through the tunnel while slab c+1 quantizes
    xl = pred_logits.reshape(B, QC)
    parts = []
    for c in range(NCORES):
        y = xl[c * BPC:(c + 1) * BPC] * QSCALE
        np.rint(y, out=y)
        np.clip(y, -127.0, 127.0, out=y)
        parts.append(jax.device_put(y.astype(np.int8), devices[c]))
    xq = jax.make_array_from_single_device_arrays(
        (B, QC), xq_sharding, parts)

    zeros = [np.zeros(s, d) for s, d in zip(zero_shapes, zero_dtypes)]
    out_arrs = sharded(xq, *zeros)

    # overlap: host small terms while the upload/exec round-trips
    ce_corr, bbox_sum, giou_sum = _host_small_terms(
        pred_logits, pred_boxes, tgt_boxes, si, tl, ew)

    r = np.asarray(out_arrs[0])                                # [B, R_N]

    anl = r[:, R_ANL0:R_ANL0 + NCH].sum(dtype=np.float64)
    aw = r[:, R_AW0:R_AW0 + NCH].sum(dtype=np.float64)
    aw2 = r[:, R_AW20:R_AW20 + NCH].sum(dtype=np.float64)
    sum_phi = -anl + 2.0 * aw - aw2                 # Sum p^2 * softplus(x)

    num_boxes = np.float32(B * Nt) + 1e-8
    ce_sum = EOS_COEF * (1.0 - ALPHA) * sum_phi + ce_corr
    loss_ce = ce_sum / num_boxes
    loss_bbox = bbox_sum / num_boxes
    loss_giou = giou_sum / num_boxes
    card = r[:, R_CARD]
    loss_card = np.abs(card - np.float32(Nt)).mean(dtype=np.float64)

    return np.array([W_CE * loss_ce, W_BBOX * loss_bbox,
                     W_GIOU * loss_giou, W_CARD * loss_card], dtype=np.float32)
